# revision 50
# baseline (speedup 1.0000x reference)
"""Self-contained Trainium2 Bass kernel for a post-LN transformer block.

Problem: y = LN(h + MLP(h)), h = LN(x + CausalAttn(x)), B=2, L=2048, D=1024,
H=16 heads, MLP hidden 4096, shared LN params, exact GELU, fp32 I/O.

Sharding (8 cores): core c handles batch b=c//4, head-group q=c%4 (heads
4q..4q+3) for attention, then rows [512q, 512q+512) of batch b for the
MLP/LN part. Two 4-rank AllToAlls (one per half of the row blocks,
replica groups {0-3} and {4-7}) reshard from column(head)-split to
row-split between the phases; round A (second half-rows) goes first so
MLP half 1 overlaps round B attention + the second collective.

Host-side prep: x is pre-transposed (xT) so no PE transposes are needed
for QKV; all weights are pre-shuffled so every DMA line is contiguous.
Matmuls run in bf16 with fp32 PSUM accumulation; residuals/LN in fp32.
"""

import contextlib
import ctypes
import sys
import types

import numpy as np

B, L, D = 2, 2048, 1024
H, HD = 16, 64
DFF = 4 * D
EPS = 1e-5
NCORES = 8
ROWS = L // 4  # 512 rows per core for MLP phase
HPC = 4  # heads per core
HCOLS = HPC * HD  # 256 attn-out cols per core
NTB = L // 128  # 16 token blocks per batch
NRB = ROWS // 128  # 4 token blocks per core row-slice


def _install_axon_hooks_shim():
    """Provide antenv.axon_hooks (NTFF profiling hook) when the image lacks it.

    Needed only when profiling (BASS_TRACE=1); harmless otherwise.
    """
    try:
        from antenv.axon_hooks import get_axon_ntff_profile_hook  # noqa: F401

        return
    except ImportError:
        pass
    try:
        import antenv
    except ImportError:
        return

    mod = types.ModuleType("antenv.axon_hooks")
    _state = {"hook": None}
    mod.set_axon_ntff_profile_hook = lambda h: _state.__setitem__("hook", h)
    mod.get_axon_ntff_profile_hook = lambda: _state["hook"]
    sys.modules["antenv.axon_hooks"] = mod
    antenv.axon_hooks = mod

    try:
        lib = ctypes.CDLL("/opt/axon/libaxon_pjrt.so")
    except OSError:
        return
    if not hasattr(lib, "axon_start_nrt_profile"):
        return
    lib.axon_start_nrt_profile.argtypes = [
        ctypes.POINTER(ctypes.c_int64),
        ctypes.c_size_t,
    ]
    lib.axon_start_nrt_profile.restype = ctypes.c_int64
    lib.axon_stop_nrt_profile.argtypes = [ctypes.c_char_p]
    lib.axon_stop_nrt_profile.restype = ctypes.c_int64

    @contextlib.contextmanager
    def _hook(output_dir, device_ids):
        import jax

        jax.devices()
        if device_ids:
            ids = (ctypes.c_int64 * len(device_ids))(*device_ids)
            rc = lib.axon_start_nrt_profile(ids, len(device_ids))
        else:
            rc = lib.axon_start_nrt_profile(None, 0)
        if rc != 0:
            raise RuntimeError(f"axon_start_nrt_profile rc={rc}")
        try:
            yield
        finally:
            n = lib.axon_stop_nrt_profile(str(output_dir).encode())
            print(f"profile: {n} file(s) -> {output_dir}", file=sys.stderr)

    mod.set_axon_ntff_profile_hook(_hook)


_install_axon_hooks_shim()

import concourse.bass as bass  # noqa: E402
import concourse.tile as tile  # noqa: E402
from concourse import bacc, mybir  # noqa: E402
from concourse.bass_utils import run_bass_kernel_spmd  # noqa: E402
from concourse.masks import make_identity  # noqa: E402

F32 = mybir.dt.float32
BF16 = mybir.dt.bfloat16
F8E4 = mybir.dt.float8e4


def _build():
    nc = bacc.Bacc(
        "TRN2", target_bir_lowering=False, debug=False, num_devices=NCORES
    )

    def din(name, shape, dt=BF16):
        return nc.dram_tensor(name, shape, dt, kind="ExternalInput").ap()

    xT = din("xT", [128, 8, L])  # x^T of this core's batch, bf16 (pre-shuffled)
    xr = din("xr", [128, NRB, D], F32)  # this core's row slice of x
    wq_c = din("wq_c", [128, 8, HCOLS])  # head-sliced, pre-scaled by 1/8
    wk_c = din("wk_c", [128, 8, HCOLS])
    wv_c = din("wv_c", [128, 8, HCOLS])
    w1s = din("w1s", [8, 128, 8, 512])  # [o4][p][ic][os] chunks, contiguous
    w2s8 = din("w2s8", [128, 16, D], F8E4)  # hc 0-15, x64, fp8
    w2s16 = din("w2s16", [128, 16, D])  # hc 16-31, x64, bf16
    mask_tri = din("mask_tri", [128, 128])  # 1 where k<=q else 0 (bf16)
    zm = din("zm", [128, 8], F32)  # 1 for same-batch a2a slots else 0
    out = nc.dram_tensor("out", [ROWS, D], F32, kind="ExternalOutput").ap()

    with tile.TileContext(nc) as tc, contextlib.ExitStack() as ctx:
        pb = ctx.enter_context(tc.tile_pool(name="pb", bufs=1))  # persistent
        pc = ctx.enter_context(tc.tile_pool(name="pc", bufs=1))  # constants
        pw = ctx.enter_context(tc.tile_pool(name="pw", bufs=1))  # resident W
        pws = ctx.enter_context(tc.tile_pool(name="pws", bufs=2))  # streamed W
        ps = ctx.enter_context(tc.tile_pool(name="ps", bufs=3))  # small tiles
        pr = ctx.enter_context(tc.tile_pool(name="pr", bufs=2))  # a2a recv
        pe = ctx.enter_context(tc.tile_pool(name="pe", bufs=3))  # exp tiles
        pp = ctx.enter_context(tc.tile_pool(name="pp", bufs=2, space="PSUM"))
        pd = ctx.enter_context(tc.tile_pool(name="pd", bufs=1, space="DRAM"))

        # ---- constants + resident weights.  DMA priority order: the first
        # QK matmul needs only wq + xT chunk 0, so those go first; w2 is not
        # needed until the MLP (~250us in) and loads last.
        ident_f = pc.tile([128, 128], F32)
        make_identity(nc, ident_f)
        eps_sb = pc.tile([128, 1], F32)
        nc.vector.memset(eps_sb, EPS)

        wq_sb = pw.tile([128, 8, HCOLS], BF16)
        nc.sync.dma_start(out=wq_sb, in_=wq_c[:, :, :])
        wk_sb = pw.tile([128, 8, HCOLS], BF16)
        nc.sync.dma_start(out=wk_sb, in_=wk_c[:, :, :])
        mask_sb = pc.tile([128, 128], BF16)
        nc.sync.dma_start(out=mask_sb, in_=mask_tri[:, :])
        wv_sb = pw.tile([128, 8, HCOLS], BF16)
        zm_sb = pc.tile([128, 8], F32)
        w2_sb8 = pw.tile([128, 16, D], F8E4)
        w2_sb16 = pw.tile([128, 16, D], BF16)

        # ---- a2a DRAM buffers (bf16 payload, two half-row rounds).
        # Slots d and d+4 both carry this core's rows for dest-chunk d (the
        # other-batch copy is junk the receiver zeroes via the zm mask).
        a2a_in1 = pd.tile([8, 256, HCOLS], BF16)
        a2a_out1 = pd.tile([8, 256, HCOLS], BF16)
        a2a_in2 = pd.tile([8, 256, HCOLS], BF16)
        a2a_out2 = pd.tile([8, 256, HCOLS], BF16)

        # ---- big SBUF tiles (tag-shared slots; lifetimes disjoint) ----
        xT_sb = pb.tile([128, 8, L], BF16, tag="slotA")  # dead after QKV
        QT = pb.tile([128, 2, L], BF16, tag="slotC")  # dead after attention
        KT = pb.tile([128, 2, L], BF16, tag="slotD")  # dead after attention
        V_ext = pb.tile([128, NTB, HPC, HD + 1], BF16, tag="slotE")
        attn_sb = pb.tile([128, NTB, HCOLS], BF16, tag="slotF")
        res1 = pb.tile([128, NRB, D], F32, tag="slotG")  # x-res + attn; then res2
        h_sb = pb.tile([128, NRB, D], F32, tag="slotH")  # LN1 output
        hT = pb.tile([128, 8, ROWS], BF16, tag="slotI")
        gT8 = pb.tile([128, 16, ROWS], F8E4, tag="slotA")  # hc 0-15, fp8
        gT16a = pb.tile([128, 8, ROWS], BF16, tag="slotC")  # hc 16-23
        gT16b = pb.tile([128, 8, ROWS], BF16, tag="slotD")  # hc 24-31

        # ---- phase 1: load xT + QKV projections, interleaved with attention.
        # xT loads split by TOKEN range (each piece spans all 8 feature
        # blocks) so the first QK contraction only waits for 512 tokens.
        for i in range(2):
            nc.sync.dma_start(
                out=xT_sb[:, :, i * 512 : (i + 1) * 512],
                in_=xT[:, :, i * 512 : (i + 1) * 512],
            )
        nc.sync.dma_start(out=wv_sb, in_=wv_c[:, :, :])
        for i in range(2, 4):
            nc.sync.dma_start(
                out=xT_sb[:, :, i * 512 : (i + 1) * 512],
                in_=xT[:, :, i * 512 : (i + 1) * 512],
            )
        nc.sync.dma_start(out=zm_sb, in_=zm[:, :])
        # res1 starts as the x residual; attn columns are added in place
        nc.sync.dma_start(out=res1, in_=xr[:, :, :])
        for i in range(4):
            nc.sync.dma_start(
                out=w2_sb8[:, 4 * i : 4 * i + 4, :],
                in_=w2s8[:, 4 * i : 4 * i + 4, :],
            )
            nc.sync.dma_start(
                out=w2_sb16[:, 4 * i : 4 * i + 4, :],
                in_=w2s16[:, 4 * i : 4 * i + 4, :],
            )
        # Preload the first two w1 chunks of MLP half 0 during attention's
        # DMA-idle window (the scalar queue is busy with exp until the end
        # of attention, and the post-a2a window is DMA-congested).
        w1c_pre = []
        for o4 in range(2):
            w1c = pws.tile(
                [128, 8, 512], BF16, tag="w1c", bufs=2, name=f"w1c_pre_{o4}"
            )
            nc.sync.dma_start(out=w1c, in_=w1s[o4])
            w1c_pre.append(w1c)


        def qk_chunk(t4):
            for oc in range(2):
                psq = pp.tile([128, 512], F32, tag="psP", bufs=2)
                for ic in range(8):
                    nc.tensor.matmul(
                        psq,
                        wq_sb[:, ic, oc * 128 : (oc + 1) * 128],
                        xT_sb[:, ic, t4 * 512 : (t4 + 1) * 512],
                        start=(ic == 0),
                        stop=(ic == 7),
                    )
                nc.vector.tensor_copy(QT[:, oc, t4 * 512 : (t4 + 1) * 512], psq)
                psk = pp.tile([128, 512], F32, tag="psP", bufs=2)
                for ic in range(8):
                    nc.tensor.matmul(
                        psk,
                        wk_sb[:, ic, oc * 128 : (oc + 1) * 128],
                        xT_sb[:, ic, t4 * 512 : (t4 + 1) * 512],
                        start=(ic == 0),
                        stop=(ic == 7),
                    )
                nc.vector.tensor_copy(KT[:, oc, t4 * 512 : (t4 + 1) * 512], psk)

        def v_chunk(tb):
            # V natural layout [tok, feat]; bv is zero in this problem
            psv = pp.tile([128, HCOLS], F32, tag="psP", bufs=2,
                          padded_shape=[128, 512], name=f"psv_{tb}")
            for ic in range(8):
                nc.tensor.matmul(
                    psv,
                    xT_sb[:, ic, tb * 128 : (tb + 1) * 128],
                    wv_sb[:, ic, :],
                    start=(ic == 0),
                    stop=(ic == 7),
                )
            for h in range(HPC):
                nc.vector.tensor_copy(
                    V_ext[:, tb, h, 0:HD], psv[:, h * HD : (h + 1) * HD]
                )

        nc.vector.memset(V_ext[:, :, :, HD : HD + 1], 1.0)

        # ---- attention: causal, scores^T layout [key_p, query_f], Lq=256 ----
        def q_slice(h, J2):
            p0 = 64 * (h % 2)
            return QT[p0 : p0 + 64, h // 2, J2 * 256 : (J2 + 1) * 256]

        def k_slice(h, k):
            p0 = 64 * (h % 2)
            return KT[p0 : p0 + 64, h // 2, k * 128 : (k + 1) * 128]

        def attn_chunk(J2):
            npairs = J2 + 1
            for h in range(HPC):
                # The two js accumulation chains interleave, so each needs its
                # own PSUM bank (start=True clears has_written bank-wide).
                psu = [
                    pp.tile([128, HD + 1], F32, tag="psU", bufs=2,
                            name=f"psu_{J2}_{h}_{js}")
                    for js in range(2)
                ]

                def attn_v(expP, g2, gp):
                    for kks in range(2 * gp):
                        k = 2 * g2 + kks
                        for js in range(2):
                            if 2 * J2 + js < k:
                                continue
                            nc.tensor.matmul(
                                psu[js],
                                expP[:, kks, js * 128 : (js + 1) * 128],
                                V_ext[:, k, h, :],
                                start=(k == 0),
                                stop=(k == 2 * J2 + js),
                            )

                # Software-pipelined: attnV for group g issues after scores
                # for group g+1, so the PE never sits behind the exp chain.
                pending = None
                for g2 in range(0, npairs, 2):  # groups of 2 kp-pairs
                    gp = min(2, npairs - g2)
                    pssP = pp.tile(
                        [128, 2 * gp, 256], F32, tag="psS", bufs=2,
                        padded_shape=[128, 4, 256], name=f"pssP_{J2}_{h}_{g2}",
                    )
                    for kps in range(gp):
                        kp = g2 + kps
                        nc.tensor.matmul(
                            pssP[:, 2 * kps, :], k_slice(h, 2 * kp),
                            q_slice(h, J2), start=True, stop=True,
                        )
                        nc.tensor.matmul(
                            pssP[:, 2 * kps + 1, :], k_slice(h, 2 * kp + 1),
                            q_slice(h, J2), start=True, stop=True,
                        )
                    expP = pe.tile([128, 2 * gp, 256], BF16, tag="expT",
                                   name=f"expP_{J2}_{h}_{g2}")
                    nc.scalar.activation(
                        expP, pssP, mybir.ActivationFunctionType.Exp
                    )
                    if g2 + gp == npairs:  # diagonal pair: causal mask inside
                        kkd = 2 * (gp - 1)
                        nc.vector.tensor_mul(
                            expP[:, kkd, 0:128], expP[:, kkd, 0:128], mask_sb
                        )
                        nc.vector.tensor_mul(
                            expP[:, kkd + 1, 128:256], expP[:, kkd + 1, 128:256],
                            mask_sb,
                        )
                    if pending is not None:
                        attn_v(*pending)
                    pending = (expP, g2, gp)
                attn_v(*pending)
                for js in range(2):
                    rec = ps.tile([128, 1], F32, tag="rec")
                    nc.vector.reciprocal(rec, psu[js][:, HD : HD + 1])
                    nc.vector.tensor_scalar_mul(
                        attn_sb[:, 2 * J2 + js, h * HD : (h + 1) * HD],
                        psu[js][:, 0:HD],
                        rec,
                    )

        def a2a_send(J2, ain):
            # gpsimd queue: keeps the collective triggers' DMA-counter waits
            # scoped to the sends alone (sync-queue counters drag in recvs).
            for dest in (J2 // 2, 4 + J2 // 2):
                nc.gpsimd.dma_start(
                    out=ain[dest].rearrange("(t p) c -> p t c", p=128),
                    in_=attn_sb[:, 2 * J2 : 2 * J2 + 2, :],
                )

        def a2a_go(ain, aout):
            nc.gpsimd.collective_compute(
                "AllToAll",
                mybir.AluOpType.bypass,
                replica_groups=[list(range(NCORES))],
                ins=[ain[:]],
                outs=[aout[:]],
            )

        # Round A (J2 even = local row blocks 0,1 of every dest — the chunks
        # whose QKV deps finish earliest) interleaves with QKV so exp/attnV
        # overlap projections; its a2a goes first so MLP half 0 can run under
        # the bigger round B + a2a #2.
        qk_chunk(0)
        v_chunk(0)
        v_chunk(1)
        attn_chunk(0)
        a2a_send(0, a2a_in1)
        v_chunk(2)
        v_chunk(3)
        qk_chunk(1)
        v_chunk(4)
        v_chunk(5)
        attn_chunk(2)
        a2a_send(2, a2a_in1)
        v_chunk(6)
        v_chunk(7)
        qk_chunk(2)
        v_chunk(8)
        v_chunk(9)
        attn_chunk(4)
        a2a_send(4, a2a_in1)
        v_chunk(10)
        v_chunk(11)
        qk_chunk(3)
        v_chunk(12)
        v_chunk(13)
        attn_chunk(6)
        a2a_send(6, a2a_in1)
        a2a_go(a2a_in1, a2a_out1)
        v_chunk(14)
        v_chunk(15)
        for J2 in (1, 3, 5, 7):
            attn_chunk(J2)
            a2a_send(J2, a2a_in2)
        a2a_go(a2a_in2, a2a_out2)

        # ---- MLP per row-half: recv+LN1+hT, m1+gelu, m2+res, LN2+out ----
        def ln_row(src_t, tb, out_ap):
            stats = ps.tile([128, 2, 6], F32, tag="stats")
            nc.vector.bn_stats(stats[:, 0, :], src_t[:, tb, 0:512])
            nc.vector.bn_stats(stats[:, 1, :], src_t[:, tb, 512:1024])
            mv = ps.tile([128, 2], F32, tag="mv")
            nc.vector.bn_aggr(mv, stats)
            std = ps.tile([128, 1], F32, tag="std")
            nc.scalar.activation(
                std, mv[:, 1:2], mybir.ActivationFunctionType.Sqrt,
                bias=eps_sb[:, 0:1], scale=1.0,
            )
            rstd = ps.tile([128, 1], F32, tag="rstd")
            nc.vector.reciprocal(rstd, std)
            # ln_g == 1, ln_b == 0 in this problem, so affine is identity
            nc.vector.tensor_scalar(
                out=out_ap,
                in0=src_t[:, tb, :],
                scalar1=mv[:, 0:1],
                scalar2=rstd,
                op0=mybir.AluOpType.subtract,
                op1=mybir.AluOpType.mult,
            )

        def stage_recv(half, aout):
            # recv + masked residual add for this half.  Slot pairs (2i,2i+1)
            # share a zm value and target adjacent column groups, so each
            # pair is one DMA + one fused multiply-add.
            t0 = 2 * half
            for i2 in range(4):
                r0 = pr.tile([128, 4, 256], BF16, tag="r0", name=f"r0_{half}_{i2}")
                nc.sync.dma_start(
                    out=r0,
                    in_=aout[2 * i2 : 2 * i2 + 2].rearrange(
                        "s (t p) c -> p (s t) c", p=128
                    ),
                )
                g = (2 * i2) % 4
                r0v = r0.rearrange("p (s t) c -> p t s c", t=2)
                for t in range(2):
                    dst = res1[
                        :, t0 + t, g * HCOLS : (g + 2) * HCOLS
                    ].rearrange("p (s c) -> p s c", s=2)
                    nc.vector.scalar_tensor_tensor(
                        out=dst,
                        in0=r0v[:, t, :, :],
                        scalar=zm_sb[:, 2 * i2 : 2 * i2 + 1],
                        in1=dst,
                        op0=mybir.AluOpType.mult,
                        op1=mybir.AluOpType.add,
                    )

        def stage_ln_t(half):
            # LN1 + hT transposes for this half
            t0, t1 = 2 * half, 2 * half + 1  # res1/h_sb row blocks
            for tb in (t0, t1):
                ln_row(res1, tb, h_sb[:, tb, :])
                for g in range(4):
                    psT = pp.tile(
                        [128, 2, 128], F32, tag="psU", bufs=2,
                        name=f"psT_{half}_{tb}_{g}",
                    )
                    for fs in range(2):
                        fc = 2 * g + fs
                        nc.tensor.transpose(
                            psT[:, fs, :], h_sb[:, tb, fc * 128 : (fc + 1) * 128],
                            ident_f,
                        )
                    nc.vector.tensor_copy(
                        hT[:, 2 * g : 2 * g + 2, tb * 128 : (tb + 1) * 128], psT
                    )

        def stage_b(half, preloaded=()):
            # m1 + gelu for this token half (b1 == 0 in this problem)
            c0 = 256 * half
            for o4 in range(8):
                if o4 < len(preloaded):
                    w1c = preloaded[o4]
                else:
                    w1c = pws.tile(
                        [128, 8, 512], BF16, tag="w1c", bufs=2,
                        name=f"w1c_{half}_{o4}",
                    )
                    # scalar queue: gpsimd is blocked by the collectives
                    nc.scalar.dma_start(out=w1c, in_=w1s[o4])
                psm = pp.tile(
                    [128, 4, 256], F32, tag="psS", bufs=2,
                    name=f"psm_{half}_{o4}",
                )
                for os_ in range(4):
                    for ic in range(8):
                        nc.tensor.matmul(
                            psm[:, os_, :],
                            w1c[:, ic, os_ * 128 : (os_ + 1) * 128],
                            hT[:, ic, c0 : c0 + 256],
                            start=(ic == 0),
                            stop=(ic == 7),
                        )
                if o4 < 4:
                    gdst = gT8[:, 4 * o4 : 4 * o4 + 4, c0 : c0 + 256]
                elif o4 < 6:
                    gdst = gT16a[:, 4 * (o4 - 4) : 4 * (o4 - 4) + 4, c0 : c0 + 256]
                else:
                    gdst = gT16b[:, 4 * (o4 - 6) : 4 * (o4 - 6) + 4, c0 : c0 + 256]
                nc.scalar.activation(
                    gdst, psm, mybir.ActivationFunctionType.Gelu,
                )

        def stage_c_mm(half):
            # m2 + residual for this half; res1 rows are dead after LN1 so
            # res2 lands there.
            t0, t1 = 2 * half, 2 * half + 1
            for tb in (t0, t1):
                for f2 in range(2):
                    pso = pp.tile(
                        [128, 512], F32, tag="psP", bufs=2,
                        name=f"pso_{half}_{tb}_{f2}",
                    )
                    for k2 in range(8):
                        nc.tensor.matmul(
                            pso,
                            gT8[:, 2 * k2 : 2 * k2 + 2, tb * 128 : (tb + 1) * 128],
                            w2_sb8[:, 2 * k2 : 2 * k2 + 2,
                                   f2 * 512 : (f2 + 1) * 512],
                            start=(k2 == 0),
                            stop=False,
                            perf_mode=mybir.MatmulPerfMode.DoubleRow,
                        )
                    for hc in range(16):
                        gt = gT16a if hc < 8 else gT16b
                        nc.tensor.matmul(
                            pso,
                            gt[:, hc % 8, tb * 128 : (tb + 1) * 128],
                            w2_sb16[:, hc, f2 * 512 : (f2 + 1) * 512],
                            start=False,
                            stop=(hc == 15),
                        )
                    # b2 == 0; w2 was pre-scaled x64 for fp8, undo here
                    nc.vector.scalar_tensor_tensor(
                        out=res1[:, tb, f2 * 512 : (f2 + 1) * 512],
                        in0=pso,
                        scalar=1.0 / 64.0,
                        in1=h_sb[:, tb, f2 * 512 : (f2 + 1) * 512],
                        op0=mybir.AluOpType.mult,
                        op1=mybir.AluOpType.add,
                    )

        def stage_out(half):
            t0, t1 = 2 * half, 2 * half + 1
            for tb in (t0, t1):
                # h_sb row block is dead after the m2 residual add — reuse it
                # as the LN2 output staging buffer.
                ln_row(res1, tb, h_sb[:, tb, :])
                nc.sync.dma_start(
                    out=out[tb * 128 : (tb + 1) * 128, :], in_=h_sb[:, tb, :]
                )

        # Stage interleave, chosen so no engine FIFO holds an instruction
        # that waits on later data than what the instructions behind it
        # need.  PE order: T0 B0 C0 T1 B1 C1.  On the vector FIFO, half 1's
        # recv-adds + LN1 (ready at a2a#2 completion) sit after half 0's
        # m2-adds but before half 0's LN2 (ready only when m2 finishes).
        stage_recv(0, a2a_out1)  # rows tb{0,1} arrived in round A
        stage_ln_t(0)
        stage_b(0, preloaded=w1c_pre)
        stage_c_mm(0)
        stage_recv(1, a2a_out2)  # rows tb{2,3} from round B
        stage_ln_t(1)
        stage_out(0)
        stage_b(1)
        # Final half: fuse LN2+output per row block into the m2 loop so the
        # first block's output drains while the second block's m2 runs.
        for tb in (2, 3):
            for f2 in range(2):
                pso = pp.tile(
                    [128, 512], F32, tag="psP", bufs=2, name=f"pso_f_{tb}_{f2}"
                )
                for k2 in range(8):
                    nc.tensor.matmul(
                        pso,
                        gT8[:, 2 * k2 : 2 * k2 + 2, tb * 128 : (tb + 1) * 128],
                        w2_sb8[:, 2 * k2 : 2 * k2 + 2, f2 * 512 : (f2 + 1) * 512],
                        start=(k2 == 0),
                        stop=False,
                        perf_mode=mybir.MatmulPerfMode.DoubleRow,
                    )
                for hc in range(16):
                    gt = gT16a if hc < 8 else gT16b
                    nc.tensor.matmul(
                        pso,
                        gt[:, hc % 8, tb * 128 : (tb + 1) * 128],
                        w2_sb16[:, hc, f2 * 512 : (f2 + 1) * 512],
                        start=False,
                        stop=(hc == 15),
                    )
                nc.vector.scalar_tensor_tensor(
                    out=res1[:, tb, f2 * 512 : (f2 + 1) * 512],
                    in0=pso,
                    scalar=1.0 / 64.0,
                    in1=h_sb[:, tb, f2 * 512 : (f2 + 1) * 512],
                    op0=mybir.AluOpType.mult,
                    op1=mybir.AluOpType.add,
                )
            ln_row(res1, tb, h_sb[:, tb, :])
            nc.sync.dma_start(
                out=out[tb * 128 : (tb + 1) * 128, :], in_=h_sb[:, tb, :]
            )

    nc.compile()
    return nc


_NC_CACHE = [None]


def kernel(**inputs) -> np.ndarray:
    import ml_dtypes

    x = np.asarray(inputs["x"], np.float32)
    wq = np.asarray(inputs["wq"], np.float32)
    wk = np.asarray(inputs["wk"], np.float32)
    wv = np.asarray(inputs["wv"], np.float32)
    w1 = np.asarray(inputs["w1"], np.float32)
    w2 = np.asarray(inputs["w2"], np.float32)

    # The kernel folds these away; setup_inputs() constructs them as
    # zeros/ones. Fail loudly if that ever changes.
    for nm in ("bq", "bk", "bv", "b1", "b2"):
        if nm in inputs:
            assert not np.any(np.asarray(inputs[nm])), f"{nm} expected zero"
    if "ln_b" in inputs:
        assert not np.any(np.asarray(inputs["ln_b"])), "ln_b expected zero"
    if "ln_g" in inputs:
        assert np.all(np.asarray(inputs["ln_g"]) == 1.0), "ln_g expected ones"

    if _NC_CACHE[0] is None:
        _NC_CACHE[0] = _build()
    nc = _NC_CACHE[0]

    bf = ml_dtypes.bfloat16
    mask = np.triu(np.ones((128, 128), np.float32)).astype(bf)
    w1sh = np.ascontiguousarray(
        w1.reshape(8, 128, 8, 512).transpose(2, 1, 0, 3)
    ).astype(bf)
    w2r = np.ascontiguousarray(w2.reshape(32, 128, D).transpose(1, 0, 2)) * 64.0
    w2sh8 = np.clip(w2r[:, :16], -240.0, 240.0).astype(ml_dtypes.float8_e4m3fn)
    w2sh16 = w2r[:, 16:].astype(bf)
    xTb = [
        np.ascontiguousarray(
            x[b].T.reshape(8, 128, L).transpose(1, 0, 2)
        ).astype(bf)
        for b in range(B)
    ]
    in_maps = []
    for c in range(NCORES):
        b, q = c // 4, c % 4
        cols = slice(HCOLS * q, HCOLS * (q + 1))
        rows = slice(ROWS * q, ROWS * (q + 1))
        zmv = np.zeros(NCORES, np.float32)
        zmv[4 * b : 4 * b + 4] = 1.0
        in_maps.append(
            {
                "xT": xTb[b],
                "xr": np.ascontiguousarray(
                    x[b, rows].reshape(NRB, 128, D).transpose(1, 0, 2)
                ),
                "wq_c": np.ascontiguousarray(
                    (wq[:, cols] * 0.125).reshape(8, 128, HCOLS).transpose(1, 0, 2)
                ).astype(bf),
                "wk_c": np.ascontiguousarray(
                    wk[:, cols].reshape(8, 128, HCOLS).transpose(1, 0, 2)
                ).astype(bf),
                "wv_c": np.ascontiguousarray(
                    wv[:, cols].reshape(8, 128, HCOLS).transpose(1, 0, 2)
                ).astype(bf),
                "w1s": w1sh,
                "w2s8": w2sh8,
                "w2s16": w2sh16,
                "mask_tri": mask,
                "zm": np.tile(zmv, (128, 1)),
            }
        )

    res = run_bass_kernel_spmd(nc, in_maps, list(range(NCORES)))
    outp = np.empty((B, L, D), np.float32)
    for c in range(NCORES):
        b, q = c // 4, c % 4
        outp[b, ROWS * q : ROWS * (q + 1)] = res.results[c]["out"]
    if getattr(res, "exec_time_ns", None) is not None:
        kernel.last_exec_time_ns = res.exec_time_ns
    return outp


kernel.last_exec_time_ns = None
